# revision 10
# baseline (speedup 1.0000x reference)
"""Dilated attention (LongNet-style) Trainium2 kernel, v4.

Problem: query/key/value (2, 8192, 12, 64) f32. Three dilation groups
(segment lengths 2048/4096/8192, dilation 1/2/4, head slices 0:4/4:8/8:12).
Each group's gather produces independent dense attention over 2048-position
dilated segments; outputs are normalized per (batch, head, channel) by the
sum over all segment positions, and divided by num_groups.

Sharding: 8 cores = 2 batches x 4 "head columns". Core c owns batch c//4 and
heads {j, 4+j, 8+j} where j = c%4 -- exactly 7 dense 2048x2048x64 attention
units per core (4 + 2 + 1 segments), perfectly balanced, with all segments of
any (batch, head) on one core so normalization needs no cross-core traffic.

Precision ("self-correcting f32r attention"): the reference's
x / x.sum(axis=(1,2)) normalization divides by a nearly-cancelling sum D,
which amplifies per-element noise ~300x -- but ONLY through D. Per-element
noise in x itself is unamplified, so the whole attention runs at reduced
precision and only D gets repaired:
  - P22 = f32r(64*exp(s)): ACT exp writes float32r (HW: round-to-nearest,
    11 mantissa bits). The PE consumes the same rounded values.
  - PV weights w22 = f32r(32*v), plus a denominator row of 32.0.
  - A matmul's cost is N cycles regardless of output partition count, so
    the 63 spare PSUM partitions of the PV matmul carry, FOR FREE, the
    w-rounding correction G[d] = sum_k (4096*(32v - w22))*P22 for channels
    0..62 (channel 63's correction is negligible in global L2 -- verified
    in simulation; 64+1+63 = 128 rows exactly fills the PSUM partition dim).
  - Host (f64): x = num/den;  D_d = sum_pos x_d + 2^-12 * sum_q r_q G[d,q];
    out = x / (3*D). The per-q r_q = 1/den_q makes G an essentially exact
    repair of the weight-rounding part of D's noise; the zero-mean P22
    rounding residual is left uncorrected (simulated total 2.9e-3 vs the
    2e-2 gate).
Scores feed the amplified path directly, so they keep k at ~fp32 via the
[kh;kl] K=128 stacking trick, with q at plain fp16 (the q-residual term was
simulated unnecessary): ONE fp16 matmul per unit. A ~160-col zero-weight
pad matmul keeps the PE stream denser than ACT's exp so the HAM clock-gate
stays at 8/8 (PE idling >~10% re-throttles the PE clock to 1.2GHz).

Device kernel per (chunk, k-block) unit (28 q-chunks of 512 x 16 k-blocks):
  S = khl_blk.T @ qhh (fp16, 512cyc) + 0-pad (160cyc) -> PSUM f32
  P22 = exp(S*ESC + ln64) -> SBUF f32r  (ACT, one batched pass per round)
  O[128, 512] += w22_blk.T @ P22 (f32r, 512cyc; rows: 64 num, 1 den, 63 G)
PE: ~2.3 streams/unit vs the 5-stream fp16-hi/lo baseline (510us). DVE only
copies chunk outputs. Engine budget per 3-unit round: PE ~1600ns (bottleneck
by design), ACT ~1490ns, DVE ~120ns.
"""

import os
import sys

if "/opt/trn_rl_repo" not in sys.path:
    sys.path.insert(0, "/opt/trn_rl_repo")
if "jax" not in sys.modules:
    os.environ.setdefault("JAX_PLATFORMS", "axon")

import math

import numpy as np

import concourse.bass as bass  # noqa: F401
import concourse.mybir as mybir
import concourse.tile as tile
from concourse import bacc
from concourse.bass_utils import run_bass_kernel_spmd

F32 = mybir.dt.float32
F32R = mybir.dt.float32r
F16 = mybir.dt.float16

B, N, H, D = 2, 8192, 12, 64
NSEG = 7           # segments per core
SEG = 2048         # dilated segment length
NCHUNK = NSEG * 4  # 512-wide q chunks per core
NKB = 16           # 128-row k blocks per segment
NUNIT = NCHUNK * NKB
RW = 3             # units per round (3 PSUM banks per score tile)
QSC = np.float32(256.0)               # fp16 pre-scale for Q/K
VSC = np.float32(32.0)                # pre-scale for V (and den row)
GSC = 4096.0                          # G-row scale: 2^12 * wres
ESC = float(0.125 / (256.0 * 256.0))  # exp scale: 1/sqrt(64) + descale
PBIAS = float(math.log(64.0))         # exp bias: P in [0.3, 16K]

_CACHE = {}
LAST_RESULT = {}


def _round_f32r(x):
    """Round f32 to the HW f32r grid (round-to-nearest, 11 mantissa bits)."""
    a = np.ascontiguousarray(x, np.float32)
    u = a.view(np.uint32).copy()
    u += np.uint32(1 << 11)
    u &= np.uint32(0xFFFFF000)
    return u.view(np.float32)


def _build_nc():
    nc = bacc.Bacc("TRN2", target_bir_lowering=False, debug=False,
                   enable_asserts=False, num_devices=8)
    qhh = nc.dram_tensor("qhh", [128, NSEG * SEG], F16, kind="ExternalInput")
    khl = nc.dram_tensor("khl", [128, NSEG * SEG], F16, kind="ExternalInput")
    w22 = nc.dram_tensor("w22", [128, NSEG * NKB * 128], F32R,
                         kind="ExternalInput")
    out = nc.dram_tensor("out", [128, NCHUNK * 512], F32,
                         kind="ExternalOutput")
    qhh_ap, khl_ap, w22_ap, out_ap = qhh.ap(), khl.ap(), w22.ap(), out.ap()

    with tile.TileContext(nc) as tc:
        with (
            tc.tile_pool(name="inp", bufs=1) as inp,
            tc.tile_pool(name="pt", bufs=4) as ptp,
            tc.tile_pool(name="osb", bufs=3) as osbp,
            tc.tile_pool(name="score", bufs=2, space="PSUM") as scp,
            tc.tile_pool(name="ot", bufs=2, space="PSUM") as otp,
        ):
            bias_t = inp.tile([128, 1], F32, tag="bias", name="bias_t")
            nc.vector.memset(bias_t[:, :], PBIAS)
            zpad = inp.tile([128, 128], F16, tag="zpad", name="zpad")
            nc.vector.memset(zpad[:, :], 0.0)

            # Warm-up prologue: runs while the input DMAs land. Dummy matmuls
            # keep the PE busy >3.4us so the HAM clock-gate opens before the
            # real rounds, and one dummy exp pulls in the ACT table load
            # (~2.7us) that would otherwise stall round 0.
            wsrc = inp.tile([128, 128], F16, tag="wsrc", name="wsrc")
            wjunk = inp.tile([128, 512], F16, tag="wjunk", name="wjunk")
            nc.vector.memset(wsrc[:, :], 0.01)
            nc.vector.memset(wjunk[:, :], 0.01)
            warm = scp.tile([128, 512 * RW], F32, tag="score", name="warm")
            for i in range(14):
                nc.tensor.matmul(warm[:, (i % 3) * 512:(i % 3 + 1) * 512],
                                 wsrc[:, :], wjunk[:, :],
                                 start=(i < 3), stop=(i >= 11))
            wp = ptp.tile([128, 512 * RW], F32R, tag="p22", name="warmp")
            nc.scalar.activation(
                wp[:, :512], warm[:, :512],
                mybir.ActivationFunctionType.Exp, scale=ESC, bias=bias_t[:, :])

            qh_sb, k_sb, w_sb = [], [], []
            for s in range(NSEG):
                qh = inp.tile([128, SEG], F16, tag=f"qh{s}", name=f"qh{s}")
                kk = inp.tile([128, SEG], F16, tag=f"k{s}", name=f"k{s}")
                wv = inp.tile([128, NKB * 128], F32R, tag=f"wv{s}",
                              name=f"wv{s}")
                # split the first segment's Q/K transfers across DMA queues so
                # round 0 isn't gated on a single ~512KB queue transfer
                nsl_dma = 4 if s == 0 else 1
                for t, ap_ in ((qh, qhh_ap), (kk, khl_ap)):
                    step = SEG // nsl_dma
                    for z in range(nsl_dma):
                        lo = z * step
                        nc.sync.dma_start(
                            t[:, lo:lo + step],
                            ap_[:, s * SEG + lo:s * SEG + lo + step])
                nc.sync.dma_start(
                    wv[:, :], w22_ap[:, s * NKB * 128:(s + 1) * NKB * 128])
                qh_sb.append(qh)
                k_sb.append(kk)
                w_sb.append(wv)

            ot_tiles = {}
            pend1, pend2 = [], []  # PV work lagged by 1 and 2 rounds

            def flush(items):
                for p22ref, i, u in items:
                    cid, kb = divmod(u, NKB)
                    s = cid // 4
                    if kb == 0:
                        ot_tiles[cid] = otp.tile([128, 512], F32, tag="ot",
                                                 name=f"ot{cid}")
                    vsl = slice(kb * 128, (kb + 1) * 128)
                    psl = slice(i * 512, (i + 1) * 512)
                    nc.tensor.matmul(ot_tiles[cid][:, :], w_sb[s][:, vsl],
                                     p22ref[:, psl],
                                     start=(kb == 0), stop=(kb == NKB - 1))
                    if kb == NKB - 1:
                        o_sb = osbp.tile([128, 512], F32, tag="osb",
                                         name=f"osb{cid}")
                        nc.vector.tensor_copy(o_sb[:, :], ot_tiles[cid][:, :])
                        nc.sync.dma_start(
                            out_ap[:, cid * 512:(cid + 1) * 512], o_sb[:, :])

            for r in range((NUNIT + RW - 1) // RW):
                units = range(r * RW, min((r + 1) * RW, NUNIT))
                nu = len(units)
                score = scp.tile([128, 512 * RW], F32, tag="score",
                                 name=f"score{r}")
                for i, u in enumerate(units):
                    cid, kb = divmod(u, NKB)
                    s, c = divmod(cid, 4)
                    osl = slice(i * 512, (i + 1) * 512)
                    csl = slice(c * 512, (c + 1) * 512)
                    ksl = slice(kb * 128, (kb + 1) * 128)
                    nc.tensor.matmul(score[:, osl], k_sb[s][:, ksl],
                                     qh_sb[s][:, csl], start=True, stop=True)
                if r < 2:
                    # startup filler: keep the PE streaming through the
                    # pipe-fill with dummies aimed at an OT-pool slot.
                    fill = otp.tile([128, 512], F32, tag="ot", name=f"fill{r}")
                    for z in range(5):
                        nc.tensor.matmul(fill[:, :], wsrc[:, :], wjunk[:, :],
                                         start=(z == 0), stop=(z == 4))
                flush(pend2)
                # single zero-weight pad after the PV work: adds 0 to the
                # last unit's score region. Keeps the PE stream above the ACT
                # exp rate (HAM clock-gate) with every LDW hidden under a
                # >=512-cycle matmul. Emitted BEFORE the exp so the WAR
                # ordering doesn't make the pad wait on the ACT.
                pz = slice((nu - 1) * 512, nu * 512)
                nc.tensor.matmul(score[:, pz], zpad[:, :],
                                 qh_sb[0][:, :512], start=False, stop=True,
                                 skip_group_check=True)
                p22 = ptp.tile([128, 512 * RW], F32R, tag="p22",
                               name=f"p22_{r}")
                nsl = slice(0, 512 * nu)
                nc.scalar.activation(
                    p22[:, nsl], score[:, nsl],
                    mybir.ActivationFunctionType.Exp, scale=ESC,
                    bias=bias_t[:, :])
                pend2 = pend1
                pend1 = [(p22, i, u) for i, u in enumerate(units)]
            flush(pend2)
            flush(pend1)

    nc.compile()
    return nc


def _gather_segs(query, key, value, core):
    b, j = divmod(core, 4)
    segs = []
    for arr in (query, key, value):
        h0 = arr[b, :, j, :].reshape(4, SEG, D)
        h1 = arr[b, :, 4 + j, :].reshape(2, 4096, D)[:, 1::2, :]
        h2 = arr[b, 2::4, 8 + j, :][None]
        segs.append(np.concatenate([h0, h1, h2], axis=0))  # [7, 2048, 64]
    return segs


def _prep_core(query, key, value, core):
    qs, ks, vs = _gather_segs(query, key, value, core)
    # [64, NSEG*SEG] with col = s*SEG + p
    qt = (qs * QSC).transpose(2, 0, 1).reshape(D, NSEG * SEG)
    kt = (ks * QSC).transpose(2, 0, 1).reshape(D, NSEG * SEG)
    qh = qt.astype(np.float16)
    kh = kt.astype(np.float16)
    kl = (kt - kh).astype(np.float16)
    v32 = (vs * VSC).astype(np.float32)            # [7, 2048, 64]
    w22v = _round_f32r(v32)
    wres = _round_f32r(GSC * (v32.astype(np.float64)
                              - w22v.astype(np.float64)).astype(np.float32))
    # per (seg, kb) block [128 kpos, 128 outrows]:
    #   cols 0:64 = w22v, col 64 = 32.0, cols 65:128 = G weights (ch 0..62)
    wblk = np.empty((NSEG, SEG, 128), np.float32)
    wblk[:, :, :64] = w22v
    wblk[:, :, 64] = float(VSC)
    wblk[:, :, 65:] = wres[:, :, :63]
    w22 = (wblk.reshape(NSEG, NKB, 128, 128).transpose(2, 0, 1, 3)
           .reshape(128, -1))
    return {
        "qhh": np.ascontiguousarray(np.concatenate([qh, qh], axis=0)),
        "khl": np.ascontiguousarray(np.concatenate([kh, kl], axis=0)),
        "w22": np.ascontiguousarray(w22),
    }


def _unshard(results, dtype):
    full = np.zeros((B, N, H, D), dtype)
    groups = [(0, 4), (4, 6), (6, 7)]
    for core in range(8):
        b, j = divmod(core, 4)
        o = results[core]["out"].astype(np.float64)
        num, den, G = o[:64], o[64], o[65:]          # [64|63, 14336], [14336]
        r = 1.0 / den
        x = num * r[None, :]
        for g0, g1 in groups:
            gcols = slice(g0 * SEG, g1 * SEG)
            Dv = x[:, gcols].sum(axis=1)             # [64]
            C = np.zeros(64)
            C[:63] = (G[:, gcols] * r[None, gcols]).sum(axis=1) / GSC
            x[:, gcols] = x[:, gcols] / (3.0 * (Dv + C))[:, None]
        h0 = x[:, :4 * SEG]
        full[b, :, j, :] = h0.T
        h1 = x[:, 4 * SEG:6 * SEG]
        for g in range(2):
            full[b, g * 4096 + 1:(g + 1) * 4096:2, 4 + j, :] = \
                h1[:, g * SEG:(g + 1) * SEG].T
        full[b, 2::4, 8 + j, :] = x[:, 6 * SEG:].T
    return full


def _consistent(results):
    for core in range(8):
        den = results[core]["out"][64].astype(np.float64)
        if not np.isfinite(den).all() or (den <= 0).any():
            return False
        # den = 32 * sum_k P22 over 2048 rows, P22 in [0.3, 16K]:
        # sane bounds catch corrupted/partial runs
        if den.min() < 32 * 2048 * 0.01 or den.max() > 32 * 2048 * 2e4:
            return False
    return True


def _ensure_axon_backend():
    """The bass PJRT path needs the axon/neuron jax backend. A harness may
    pin JAX_PLATFORMS=cpu for its reference; re-select axon if so."""
    import jax
    try:
        plat = jax.devices()[0].platform
    except Exception:
        plat = ""
    if plat not in ("axon", "neuron"):
        try:
            jax.config.update("jax_platforms", "axon,cpu")
            jax.devices()
        except Exception:
            pass


def kernel(query, key, value):
    _ensure_axon_backend()
    query = np.asarray(query, np.float32)
    key = np.asarray(key, np.float32)
    value = np.asarray(value, np.float32)
    assert query.shape == (B, N, H, D)

    if "nc" not in _CACHE:
        _CACHE["nc"] = _build_nc()
    nc = _CACHE["nc"]

    in_maps = [_prep_core(query, key, value, c) for c in range(8)]
    res = run_bass_kernel_spmd(nc, in_maps, core_ids=list(range(8)))
    if not _consistent(res.results):
        # transient first-execution flakes have been observed once; both
        # checks can only fail on a corrupted run, so retry once.
        res = run_bass_kernel_spmd(nc, in_maps, core_ids=list(range(8)))
    LAST_RESULT["exec_time_ns"] = res.exec_time_ns
    LAST_RESULT["results"] = res.results
    return _unshard(res.results, query.dtype)


# revision 11
# speedup vs baseline: 1.1547x; 1.1547x over previous
"""Dilated attention (LongNet-style) Trainium2 kernel, v4.

Problem: query/key/value (2, 8192, 12, 64) f32. Three dilation groups
(segment lengths 2048/4096/8192, dilation 1/2/4, head slices 0:4/4:8/8:12).
Each group's gather produces independent dense attention over 2048-position
dilated segments; outputs are normalized per (batch, head, channel) by the
sum over all segment positions, and divided by num_groups.

Sharding: 8 cores = 2 batches x 4 "head columns". Core c owns batch c//4 and
heads {j, 4+j, 8+j} where j = c%4 -- exactly 7 dense 2048x2048x64 attention
units per core (4 + 2 + 1 segments), perfectly balanced, with all segments of
any (batch, head) on one core so normalization needs no cross-core traffic.

Precision ("self-correcting f32r attention"): the reference's
x / x.sum(axis=(1,2)) normalization divides by a nearly-cancelling sum D,
which amplifies per-element noise ~300x -- but ONLY through D. Per-element
noise in x itself is unamplified, so the whole attention runs at reduced
precision and only D gets repaired:
  - P22 = f32r(64*exp(s)): ACT exp writes float32r (HW: round-to-nearest,
    11 mantissa bits). The PE consumes the same rounded values.
  - PV weights w22 = f32r(32*v), plus a denominator row of 32.0.
  - A matmul's cost is N cycles regardless of output partition count, so
    the 63 spare PSUM partitions of the PV matmul carry, FOR FREE, the
    w-rounding correction G[d] = sum_k (4096*(32v - w22))*P22 for channels
    0..62 (channel 63's correction is negligible in global L2 -- verified
    in simulation; 64+1+63 = 128 rows exactly fills the PSUM partition dim).
  - Host (f64): x = num/den;  D_d = sum_pos x_d + 2^-12 * sum_q r_q G[d,q];
    out = x / (3*D). The per-q r_q = 1/den_q makes G an essentially exact
    repair of the weight-rounding part of D's noise; the zero-mean P22
    rounding residual is left uncorrected (simulated total 2.9e-3 vs the
    2e-2 gate).
Scores feed the amplified path directly, so they keep k at ~fp32 via the
[kh;kl] K=128 stacking trick, with q at plain fp16 (the q-residual term was
simulated unnecessary): ONE fp16 matmul per unit. A ~160-col zero-weight
pad matmul keeps the PE stream denser than ACT's exp so the HAM clock-gate
stays at 8/8 (PE idling >~10% re-throttles the PE clock to 1.2GHz).

Device kernel per (chunk, k-block) unit (28 q-chunks of 512 x 16 k-blocks):
  S = khl_blk.T @ qhh (fp16, 512cyc) + 0-pad (160cyc) -> PSUM f32
  P22 = exp(S*ESC + ln64) -> SBUF f32r  (ACT, one batched pass per round)
  O[128, 512] += w22_blk.T @ P22 (f32r, 512cyc; rows: 64 num, 1 den, 63 G)
PE: ~2.3 streams/unit vs the 5-stream fp16-hi/lo baseline (510us). DVE only
copies chunk outputs. Engine budget per 3-unit round: PE ~1600ns (bottleneck
by design), ACT ~1490ns, DVE ~120ns.
"""

import os
import sys

if "/opt/trn_rl_repo" not in sys.path:
    sys.path.insert(0, "/opt/trn_rl_repo")
if "jax" not in sys.modules:
    os.environ.setdefault("JAX_PLATFORMS", "axon")

import math

import numpy as np

import concourse.bass as bass  # noqa: F401
import concourse.mybir as mybir
import concourse.tile as tile
from concourse import bacc
from concourse.bass_utils import run_bass_kernel_spmd

F32 = mybir.dt.float32
F32R = mybir.dt.float32r
F16 = mybir.dt.float16

B, N, H, D = 2, 8192, 12, 64
NSEG = 7           # segments per core
SEG = 2048         # dilated segment length
NCHUNK = NSEG * 4  # 512-wide q chunks per core
NKB = 16           # 128-row k blocks per segment
NUNIT = NCHUNK * NKB
RW = 3             # units per round (3 PSUM banks per score tile)
QSC = np.float32(256.0)               # fp16 pre-scale for Q/K
VSC = np.float32(32.0)                # pre-scale for V (and den row)
GSC = 4096.0                          # G-row scale: 2^12 * wres
ESC = float(0.125 / (256.0 * 256.0))  # exp scale: 1/sqrt(64) + descale
PBIAS = float(math.log(64.0))         # exp bias: P in [0.3, 16K]

_CACHE = {}
LAST_RESULT = {}


def _round_f32r(x):
    """Round f32 to the HW f32r grid (round-to-nearest, 11 mantissa bits)."""
    a = np.ascontiguousarray(x, np.float32)
    u = a.view(np.uint32).copy()
    u += np.uint32(1 << 11)
    u &= np.uint32(0xFFFFF000)
    return u.view(np.float32)


def _build_nc():
    nc = bacc.Bacc("TRN2", target_bir_lowering=False, debug=False,
                   enable_asserts=False, num_devices=8)
    qhh = nc.dram_tensor("qhh", [128, NSEG * SEG], F16, kind="ExternalInput")
    khl = nc.dram_tensor("khl", [128, NSEG * SEG], F16, kind="ExternalInput")
    w22 = nc.dram_tensor("w22", [128, NSEG * NKB * 128], F32R,
                         kind="ExternalInput")
    out = nc.dram_tensor("out", [128, NCHUNK * 512], F32,
                         kind="ExternalOutput")
    qhh_ap, khl_ap, w22_ap, out_ap = qhh.ap(), khl.ap(), w22.ap(), out.ap()

    with tile.TileContext(nc) as tc:
        with (
            tc.tile_pool(name="inp", bufs=1) as inp,
            tc.tile_pool(name="pt", bufs=4) as ptp,
            tc.tile_pool(name="osb", bufs=3) as osbp,
            tc.tile_pool(name="score", bufs=2, space="PSUM") as scp,
            tc.tile_pool(name="ot", bufs=2, space="PSUM") as otp,
        ):
            bias_t = inp.tile([128, 1], F32, tag="bias", name="bias_t")
            nc.vector.memset(bias_t[:, :], PBIAS)
            zpad = inp.tile([128, 128], F16, tag="zpad", name="zpad")
            nc.vector.memset(zpad[:, :], 0.0)

            # Warm-up prologue: runs while the input DMAs land. Dummy matmuls
            # keep the PE busy >3.4us so the HAM clock-gate opens before the
            # real rounds, and one dummy exp pulls in the ACT table load
            # (~2.7us) that would otherwise stall round 0.
            wsrc = inp.tile([128, 128], F16, tag="wsrc", name="wsrc")
            wjunk = inp.tile([128, 512], F16, tag="wjunk", name="wjunk")
            nc.vector.memset(wsrc[:, :], 0.01)
            nc.vector.memset(wjunk[:, :], 0.01)
            warm = scp.tile([128, 512 * RW], F32, tag="score", name="warm")
            for i in range(14):
                nc.tensor.matmul(warm[:, (i % 3) * 512:(i % 3 + 1) * 512],
                                 wsrc[:, :], wjunk[:, :],
                                 start=(i < 3), stop=(i >= 11))
            wp = ptp.tile([128, 512 * RW], F32R, tag="p22", name="warmp")
            nc.scalar.activation(
                wp[:, :512], warm[:, :512],
                mybir.ActivationFunctionType.Exp, scale=ESC, bias=bias_t[:, :])

            qh_sb, k_sb, w_sb = [], [], []
            for s in range(NSEG):
                qh = inp.tile([128, SEG], F16, tag=f"qh{s}", name=f"qh{s}")
                kk = inp.tile([128, SEG], F16, tag=f"k{s}", name=f"k{s}")
                wv = inp.tile([128, NKB * 128], F32R, tag=f"wv{s}",
                              name=f"wv{s}")
                # split the first segment's Q/K transfers across DMA queues so
                # round 0 isn't gated on a single ~512KB queue transfer
                nsl_dma = 4 if s == 0 else 1
                for t, ap_ in ((qh, qhh_ap), (kk, khl_ap)):
                    step = SEG // nsl_dma
                    for z in range(nsl_dma):
                        lo = z * step
                        nc.sync.dma_start(
                            t[:, lo:lo + step],
                            ap_[:, s * SEG + lo:s * SEG + lo + step])
                nc.sync.dma_start(
                    wv[:, :], w22_ap[:, s * NKB * 128:(s + 1) * NKB * 128])
                qh_sb.append(qh)
                k_sb.append(kk)
                w_sb.append(wv)

            ot_tiles = {}
            pend1, pend2 = [], []  # PV work lagged by 1 and 2 rounds

            def flush(items):
                for p22ref, i, u in items:
                    cid, kb = divmod(u, NKB)
                    s = cid // 4
                    if kb == 0:
                        ot_tiles[cid] = otp.tile([128, 512], F32, tag="ot",
                                                 name=f"ot{cid}")
                    vsl = slice(kb * 128, (kb + 1) * 128)
                    psl = slice(i * 512, (i + 1) * 512)
                    nc.tensor.matmul(ot_tiles[cid][:, :], w_sb[s][:, vsl],
                                     p22ref[:, psl],
                                     start=(kb == 0), stop=(kb == NKB - 1))
                    if kb == NKB - 1:
                        o_sb = osbp.tile([128, 512], F32, tag="osb",
                                         name=f"osb{cid}")
                        nc.vector.tensor_copy(o_sb[:, :], ot_tiles[cid][:, :])
                        nc.sync.dma_start(
                            out_ap[:, cid * 512:(cid + 1) * 512], o_sb[:, :])

            for r in range((NUNIT + RW - 1) // RW):
                units = range(r * RW, min((r + 1) * RW, NUNIT))
                nu = len(units)
                score = scp.tile([128, 512 * RW], F32, tag="score",
                                 name=f"score{r}")
                for i, u in enumerate(units):
                    cid, kb = divmod(u, NKB)
                    s, c = divmod(cid, 4)
                    osl = slice(i * 512, (i + 1) * 512)
                    csl = slice(c * 512, (c + 1) * 512)
                    ksl = slice(kb * 128, (kb + 1) * 128)
                    nc.tensor.matmul(score[:, osl], k_sb[s][:, ksl],
                                     qh_sb[s][:, csl], start=True, stop=True)
                p22 = ptp.tile([128, 512 * RW], F32R, tag="p22",
                               name=f"p22_{r}")
                nsl = slice(0, 512 * nu)
                nc.scalar.activation(
                    p22[:, nsl], score[:, nsl],
                    mybir.ActivationFunctionType.Exp, scale=ESC,
                    bias=bias_t[:, :])
                if r < 2:
                    # startup filler: keep the PE streaming through the
                    # pipe-fill with dummies aimed at an OT-pool slot.
                    fill = otp.tile([128, 512], F32, tag="ot", name=f"fill{r}")
                    for z in range(5):
                        nc.tensor.matmul(fill[:, :], wsrc[:, :], wjunk[:, :],
                                         start=(z == 0), stop=(z == 4))
                flush(pend2)
                pend2 = pend1
                pend1 = [(p22, i, u) for i, u in enumerate(units)]
            flush(pend2)
            flush(pend1)

    nc.compile()
    return nc


def _gather_segs(query, key, value, core):
    b, j = divmod(core, 4)
    segs = []
    for arr in (query, key, value):
        h0 = arr[b, :, j, :].reshape(4, SEG, D)
        h1 = arr[b, :, 4 + j, :].reshape(2, 4096, D)[:, 1::2, :]
        h2 = arr[b, 2::4, 8 + j, :][None]
        segs.append(np.concatenate([h0, h1, h2], axis=0))  # [7, 2048, 64]
    return segs


def _prep_core(query, key, value, core):
    qs, ks, vs = _gather_segs(query, key, value, core)
    # [64, NSEG*SEG] with col = s*SEG + p
    qt = (qs * QSC).transpose(2, 0, 1).reshape(D, NSEG * SEG)
    kt = (ks * QSC).transpose(2, 0, 1).reshape(D, NSEG * SEG)
    qh = qt.astype(np.float16)
    kh = kt.astype(np.float16)
    kl = (kt - kh).astype(np.float16)
    v32 = (vs * VSC).astype(np.float32)            # [7, 2048, 64]
    w22v = _round_f32r(v32)
    wres = _round_f32r(GSC * (v32.astype(np.float64)
                              - w22v.astype(np.float64)).astype(np.float32))
    # per (seg, kb) block [128 kpos, 128 outrows]:
    #   cols 0:64 = w22v, col 64 = 32.0, cols 65:128 = G weights (ch 0..62)
    wblk = np.empty((NSEG, SEG, 128), np.float32)
    wblk[:, :, :64] = w22v
    wblk[:, :, 64] = float(VSC)
    wblk[:, :, 65:] = wres[:, :, :63]
    w22 = (wblk.reshape(NSEG, NKB, 128, 128).transpose(2, 0, 1, 3)
           .reshape(128, -1))
    return {
        "qhh": np.ascontiguousarray(np.concatenate([qh, qh], axis=0)),
        "khl": np.ascontiguousarray(np.concatenate([kh, kl], axis=0)),
        "w22": np.ascontiguousarray(w22),
    }


def _unshard(results, dtype):
    full = np.zeros((B, N, H, D), dtype)
    groups = [(0, 4), (4, 6), (6, 7)]
    for core in range(8):
        b, j = divmod(core, 4)
        o = results[core]["out"].astype(np.float64)
        num, den, G = o[:64], o[64], o[65:]          # [64|63, 14336], [14336]
        r = 1.0 / den
        x = num * r[None, :]
        for g0, g1 in groups:
            gcols = slice(g0 * SEG, g1 * SEG)
            Dv = x[:, gcols].sum(axis=1)             # [64]
            C = np.zeros(64)
            C[:63] = (G[:, gcols] * r[None, gcols]).sum(axis=1) / GSC
            x[:, gcols] = x[:, gcols] / (3.0 * (Dv + C))[:, None]
        h0 = x[:, :4 * SEG]
        full[b, :, j, :] = h0.T
        h1 = x[:, 4 * SEG:6 * SEG]
        for g in range(2):
            full[b, g * 4096 + 1:(g + 1) * 4096:2, 4 + j, :] = \
                h1[:, g * SEG:(g + 1) * SEG].T
        full[b, 2::4, 8 + j, :] = x[:, 6 * SEG:].T
    return full


def _consistent(results):
    for core in range(8):
        den = results[core]["out"][64].astype(np.float64)
        if not np.isfinite(den).all() or (den <= 0).any():
            return False
        # den = 32 * sum_k P22 over 2048 rows, P22 in [0.3, 16K]:
        # sane bounds catch corrupted/partial runs
        if den.min() < 32 * 2048 * 0.01 or den.max() > 32 * 2048 * 2e4:
            return False
    return True


def _ensure_axon_backend():
    """The bass PJRT path needs the axon/neuron jax backend. A harness may
    pin JAX_PLATFORMS=cpu for its reference; re-select axon if so."""
    import jax
    try:
        plat = jax.devices()[0].platform
    except Exception:
        plat = ""
    if plat not in ("axon", "neuron"):
        try:
            jax.config.update("jax_platforms", "axon,cpu")
            jax.devices()
        except Exception:
            pass


def kernel(query, key, value):
    _ensure_axon_backend()
    query = np.asarray(query, np.float32)
    key = np.asarray(key, np.float32)
    value = np.asarray(value, np.float32)
    assert query.shape == (B, N, H, D)

    if "nc" not in _CACHE:
        _CACHE["nc"] = _build_nc()
    nc = _CACHE["nc"]

    in_maps = [_prep_core(query, key, value, c) for c in range(8)]
    res = run_bass_kernel_spmd(nc, in_maps, core_ids=list(range(8)))
    if not _consistent(res.results):
        # transient first-execution flakes have been observed once; both
        # checks can only fail on a corrupted run, so retry once.
        res = run_bass_kernel_spmd(nc, in_maps, core_ids=list(range(8)))
    LAST_RESULT["exec_time_ns"] = res.exec_time_ns
    LAST_RESULT["results"] = res.results
    return _unshard(res.results, query.dtype)


# revision 12
# speedup vs baseline: 1.1575x; 1.0025x over previous
"""Dilated attention (LongNet-style) Trainium2 kernel, v4.

Problem: query/key/value (2, 8192, 12, 64) f32. Three dilation groups
(segment lengths 2048/4096/8192, dilation 1/2/4, head slices 0:4/4:8/8:12).
Each group's gather produces independent dense attention over 2048-position
dilated segments; outputs are normalized per (batch, head, channel) by the
sum over all segment positions, and divided by num_groups.

Sharding: 8 cores = 2 batches x 4 "head columns". Core c owns batch c//4 and
heads {j, 4+j, 8+j} where j = c%4 -- exactly 7 dense 2048x2048x64 attention
units per core (4 + 2 + 1 segments), perfectly balanced, with all segments of
any (batch, head) on one core so normalization needs no cross-core traffic.

Precision ("self-correcting f32r attention"): the reference's
x / x.sum(axis=(1,2)) normalization divides by a nearly-cancelling sum D,
which amplifies per-element noise ~300x -- but ONLY through D. Per-element
noise in x itself is unamplified, so the whole attention runs at reduced
precision and only D gets repaired:
  - P22 = f32r(64*exp(s)): ACT exp writes float32r (HW: round-to-nearest,
    11 mantissa bits). The PE consumes the same rounded values.
  - PV weights w22 = f32r(32*v), plus a denominator row of 32.0.
  - A matmul's cost is N cycles regardless of output partition count, so
    the 63 spare PSUM partitions of the PV matmul carry, FOR FREE, the
    w-rounding correction G[d] = sum_k (4096*(32v - w22))*P22 for channels
    0..62 (channel 63's correction is negligible in global L2 -- verified
    in simulation; 64+1+63 = 128 rows exactly fills the PSUM partition dim).
  - Host (f64): x = num/den;  D_d = sum_pos x_d + 2^-12 * sum_q r_q G[d,q];
    out = x / (3*D). The per-q r_q = 1/den_q makes G an essentially exact
    repair of the weight-rounding part of D's noise; the zero-mean P22
    rounding residual is left uncorrected (simulated total 2.9e-3 vs the
    2e-2 gate).
Scores feed the amplified path directly, so they keep k at ~fp32 via the
[kh;kl] K=128 stacking trick, with q at plain fp16 (the q-residual term was
simulated unnecessary): ONE fp16 matmul per unit. A ~160-col zero-weight
pad matmul keeps the PE stream denser than ACT's exp so the HAM clock-gate
stays at 8/8 (PE idling >~10% re-throttles the PE clock to 1.2GHz).

Device kernel per (chunk, k-block) unit (28 q-chunks of 512 x 16 k-blocks):
  S = khl_blk.T @ qhh (fp16, 512cyc) + 0-pad (160cyc) -> PSUM f32
  P22 = exp(S*ESC + ln64) -> SBUF f32r  (ACT, one batched pass per round)
  O[128, 512] += w22_blk.T @ P22 (f32r, 512cyc; rows: 64 num, 1 den, 63 G)
PE: ~2.3 streams/unit vs the 5-stream fp16-hi/lo baseline (510us). DVE only
copies chunk outputs. Engine budget per 3-unit round: PE ~1600ns (bottleneck
by design), ACT ~1490ns, DVE ~120ns.
"""

import os
import sys

if "/opt/trn_rl_repo" not in sys.path:
    sys.path.insert(0, "/opt/trn_rl_repo")
if "jax" not in sys.modules:
    os.environ.setdefault("JAX_PLATFORMS", "axon")

import math

import numpy as np

import concourse.bass as bass  # noqa: F401
import concourse.mybir as mybir
import concourse.tile as tile
from concourse import bacc
from concourse.bass_utils import run_bass_kernel_spmd

F32 = mybir.dt.float32
F32R = mybir.dt.float32r
F16 = mybir.dt.float16

B, N, H, D = 2, 8192, 12, 64
NSEG = 7           # segments per core
SEG = 2048         # dilated segment length
NCHUNK = NSEG * 4  # 512-wide q chunks per core
NKB = 16           # 128-row k blocks per segment
NUNIT = NCHUNK * NKB
RW = 3             # units per round (3 PSUM banks per score tile)
QSC = np.float32(256.0)               # fp16 pre-scale for Q/K
VSC = np.float32(32.0)                # pre-scale for V (and den row)
GSC = 4096.0                          # G-row scale: 2^12 * wres
ESC = float(0.125 / (256.0 * 256.0))  # exp scale: 1/sqrt(64) + descale
PBIAS = float(math.log(64.0))         # exp bias: P in [0.3, 16K]

_CACHE = {}
LAST_RESULT = {}


def _round_f32r(x):
    """Round f32 to the HW f32r grid (round-to-nearest, 11 mantissa bits)."""
    a = np.ascontiguousarray(x, np.float32)
    u = a.view(np.uint32).copy()
    u += np.uint32(1 << 11)
    u &= np.uint32(0xFFFFF000)
    return u.view(np.float32)


def _build_nc():
    nc = bacc.Bacc("TRN2", target_bir_lowering=False, debug=False,
                   enable_asserts=False, num_devices=8)
    qhh = nc.dram_tensor("qhh", [128, NSEG * SEG], F16, kind="ExternalInput")
    khl = nc.dram_tensor("khl", [128, NSEG * SEG], F16, kind="ExternalInput")
    w22 = nc.dram_tensor("w22", [128, NSEG * NKB * 128], F32R,
                         kind="ExternalInput")
    out = nc.dram_tensor("out", [128, NCHUNK * 512], F32,
                         kind="ExternalOutput")
    qhh_ap, khl_ap, w22_ap, out_ap = qhh.ap(), khl.ap(), w22.ap(), out.ap()

    with tile.TileContext(nc) as tc:
        with (
            tc.tile_pool(name="inp", bufs=1) as inp,
            tc.tile_pool(name="pt", bufs=5) as ptp,
            tc.tile_pool(name="osb", bufs=6) as osbp,
            tc.tile_pool(name="score", bufs=2, space="PSUM") as scp,
            tc.tile_pool(name="ot", bufs=2, space="PSUM") as otp,
        ):
            bias_t = inp.tile([128, 1], F32, tag="bias", name="bias_t")
            nc.vector.memset(bias_t[:, :], PBIAS)
            zpad = inp.tile([128, 128], F16, tag="zpad", name="zpad")
            nc.vector.memset(zpad[:, :], 0.0)

            # Warm-up prologue: runs while the input DMAs land. Dummy matmuls
            # keep the PE busy >3.4us so the HAM clock-gate opens before the
            # real rounds, and one dummy exp pulls in the ACT table load
            # (~2.7us) that would otherwise stall round 0.
            wsrc = inp.tile([128, 128], F16, tag="wsrc", name="wsrc")
            wjunk = inp.tile([128, 512], F16, tag="wjunk", name="wjunk")
            nc.vector.memset(wsrc[:, :], 0.01)
            nc.vector.memset(wjunk[:, :], 0.01)
            warm = scp.tile([128, 512 * RW], F32, tag="score", name="warm")
            for i in range(14):
                nc.tensor.matmul(warm[:, (i % 3) * 512:(i % 3 + 1) * 512],
                                 wsrc[:, :], wjunk[:, :],
                                 start=(i < 3), stop=(i >= 11))
            wp = ptp.tile([128, 512 * RW], F32R, tag="p22", name="warmp")
            nc.scalar.activation(
                wp[:, :512], warm[:, :512],
                mybir.ActivationFunctionType.Exp, scale=ESC, bias=bias_t[:, :])

            qh_sb, k_sb, w_sb = [], [], []
            for s in range(NSEG):
                qh = inp.tile([128, SEG], F16, tag=f"qh{s}", name=f"qh{s}")
                kk = inp.tile([128, SEG], F16, tag=f"k{s}", name=f"k{s}")
                wv = inp.tile([128, NKB * 128], F32R, tag=f"wv{s}",
                              name=f"wv{s}")
                # split the first segment's Q/K transfers across DMA queues so
                # round 0 isn't gated on a single ~512KB queue transfer
                nsl_dma = 4 if s == 0 else 1
                for t, ap_ in ((qh, qhh_ap), (kk, khl_ap)):
                    step = SEG // nsl_dma
                    for z in range(nsl_dma):
                        lo = z * step
                        nc.sync.dma_start(
                            t[:, lo:lo + step],
                            ap_[:, s * SEG + lo:s * SEG + lo + step])
                nc.sync.dma_start(
                    wv[:, :], w22_ap[:, s * NKB * 128:(s + 1) * NKB * 128])
                qh_sb.append(qh)
                k_sb.append(kk)
                w_sb.append(wv)

            ot_tiles = {}
            pend1, pend2 = [], []  # PV work lagged by 1 and 2 rounds

            def flush(items):
                for p22ref, i, u in items:
                    cid, kb = divmod(u, NKB)
                    s = cid // 4
                    if kb == 0:
                        ot_tiles[cid] = otp.tile([128, 512], F32, tag="ot",
                                                 name=f"ot{cid}")
                    vsl = slice(kb * 128, (kb + 1) * 128)
                    psl = slice(i * 512, (i + 1) * 512)
                    nc.tensor.matmul(ot_tiles[cid][:, :], w_sb[s][:, vsl],
                                     p22ref[:, psl],
                                     start=(kb == 0), stop=(kb == NKB - 1))
                    if kb == NKB - 1:
                        o_sb = osbp.tile([128, 512], F32, tag="osb",
                                         name=f"osb{cid}")
                        nc.vector.tensor_copy(o_sb[:, :], ot_tiles[cid][:, :])
                        nc.sync.dma_start(
                            out_ap[:, cid * 512:(cid + 1) * 512], o_sb[:, :])

            for r in range((NUNIT + RW - 1) // RW):
                units = range(r * RW, min((r + 1) * RW, NUNIT))
                nu = len(units)
                score = scp.tile([128, 512 * RW], F32, tag="score",
                                 name=f"score{r}")
                for i, u in enumerate(units):
                    cid, kb = divmod(u, NKB)
                    s, c = divmod(cid, 4)
                    osl = slice(i * 512, (i + 1) * 512)
                    csl = slice(c * 512, (c + 1) * 512)
                    ksl = slice(kb * 128, (kb + 1) * 128)
                    nc.tensor.matmul(score[:, osl], k_sb[s][:, ksl],
                                     qh_sb[s][:, csl], start=True, stop=True)
                p22 = ptp.tile([128, 512 * RW], F32R, tag="p22",
                               name=f"p22_{r}")
                nsl = slice(0, 512 * nu)
                nc.scalar.activation(
                    p22[:, nsl], score[:, nsl],
                    mybir.ActivationFunctionType.Exp, scale=ESC,
                    bias=bias_t[:, :])
                if r < 2:
                    # startup filler: keep the PE streaming through the
                    # pipe-fill with dummies aimed at an OT-pool slot.
                    fill = otp.tile([128, 512], F32, tag="ot", name=f"fill{r}")
                    for z in range(5):
                        nc.tensor.matmul(fill[:, :], wsrc[:, :], wjunk[:, :],
                                         start=(z == 0), stop=(z == 4))
                flush(pend2)
                pend2 = pend1
                pend1 = [(p22, i, u) for i, u in enumerate(units)]
            flush(pend2)
            flush(pend1)

    nc.compile()
    return nc


def _gather_segs(query, key, value, core):
    b, j = divmod(core, 4)
    segs = []
    for arr in (query, key, value):
        h0 = arr[b, :, j, :].reshape(4, SEG, D)
        h1 = arr[b, :, 4 + j, :].reshape(2, 4096, D)[:, 1::2, :]
        h2 = arr[b, 2::4, 8 + j, :][None]
        segs.append(np.concatenate([h0, h1, h2], axis=0))  # [7, 2048, 64]
    return segs


def _prep_core(query, key, value, core):
    qs, ks, vs = _gather_segs(query, key, value, core)
    # [64, NSEG*SEG] with col = s*SEG + p
    qt = (qs * QSC).transpose(2, 0, 1).reshape(D, NSEG * SEG)
    kt = (ks * QSC).transpose(2, 0, 1).reshape(D, NSEG * SEG)
    qh = qt.astype(np.float16)
    kh = kt.astype(np.float16)
    kl = (kt - kh).astype(np.float16)
    v32 = (vs * VSC).astype(np.float32)            # [7, 2048, 64]
    w22v = _round_f32r(v32)
    wres = _round_f32r(GSC * (v32.astype(np.float64)
                              - w22v.astype(np.float64)).astype(np.float32))
    # per (seg, kb) block [128 kpos, 128 outrows]:
    #   cols 0:64 = w22v, col 64 = 32.0, cols 65:128 = G weights (ch 0..62)
    wblk = np.empty((NSEG, SEG, 128), np.float32)
    wblk[:, :, :64] = w22v
    wblk[:, :, 64] = float(VSC)
    wblk[:, :, 65:] = wres[:, :, :63]
    w22 = (wblk.reshape(NSEG, NKB, 128, 128).transpose(2, 0, 1, 3)
           .reshape(128, -1))
    return {
        "qhh": np.ascontiguousarray(np.concatenate([qh, qh], axis=0)),
        "khl": np.ascontiguousarray(np.concatenate([kh, kl], axis=0)),
        "w22": np.ascontiguousarray(w22),
    }


def _unshard(results, dtype):
    full = np.zeros((B, N, H, D), dtype)
    groups = [(0, 4), (4, 6), (6, 7)]
    for core in range(8):
        b, j = divmod(core, 4)
        o = results[core]["out"].astype(np.float64)
        num, den, G = o[:64], o[64], o[65:]          # [64|63, 14336], [14336]
        r = 1.0 / den
        x = num * r[None, :]
        for g0, g1 in groups:
            gcols = slice(g0 * SEG, g1 * SEG)
            Dv = x[:, gcols].sum(axis=1)             # [64]
            C = np.zeros(64)
            C[:63] = (G[:, gcols] * r[None, gcols]).sum(axis=1) / GSC
            x[:, gcols] = x[:, gcols] / (3.0 * (Dv + C))[:, None]
        h0 = x[:, :4 * SEG]
        full[b, :, j, :] = h0.T
        h1 = x[:, 4 * SEG:6 * SEG]
        for g in range(2):
            full[b, g * 4096 + 1:(g + 1) * 4096:2, 4 + j, :] = \
                h1[:, g * SEG:(g + 1) * SEG].T
        full[b, 2::4, 8 + j, :] = x[:, 6 * SEG:].T
    return full


def _consistent(results):
    for core in range(8):
        den = results[core]["out"][64].astype(np.float64)
        if not np.isfinite(den).all() or (den <= 0).any():
            return False
        # den = 32 * sum_k P22 over 2048 rows, P22 in [0.3, 16K]:
        # sane bounds catch corrupted/partial runs
        if den.min() < 32 * 2048 * 0.01 or den.max() > 32 * 2048 * 2e4:
            return False
    return True


def _ensure_axon_backend():
    """The bass PJRT path needs the axon/neuron jax backend. A harness may
    pin JAX_PLATFORMS=cpu for its reference; re-select axon if so."""
    import jax
    try:
        plat = jax.devices()[0].platform
    except Exception:
        plat = ""
    if plat not in ("axon", "neuron"):
        try:
            jax.config.update("jax_platforms", "axon,cpu")
            jax.devices()
        except Exception:
            pass


def kernel(query, key, value):
    _ensure_axon_backend()
    query = np.asarray(query, np.float32)
    key = np.asarray(key, np.float32)
    value = np.asarray(value, np.float32)
    assert query.shape == (B, N, H, D)

    if "nc" not in _CACHE:
        _CACHE["nc"] = _build_nc()
    nc = _CACHE["nc"]

    in_maps = [_prep_core(query, key, value, c) for c in range(8)]
    res = run_bass_kernel_spmd(nc, in_maps, core_ids=list(range(8)))
    if not _consistent(res.results):
        # transient first-execution flakes have been observed once; both
        # checks can only fail on a corrupted run, so retry once.
        res = run_bass_kernel_spmd(nc, in_maps, core_ids=list(range(8)))
    LAST_RESULT["exec_time_ns"] = res.exec_time_ns
    LAST_RESULT["results"] = res.results
    return _unshard(res.results, query.dtype)


# revision 14
# speedup vs baseline: 1.1763x; 1.0162x over previous
"""Dilated attention (LongNet-style) Trainium2 kernel, v4.

Problem: query/key/value (2, 8192, 12, 64) f32. Three dilation groups
(segment lengths 2048/4096/8192, dilation 1/2/4, head slices 0:4/4:8/8:12).
Each group's gather produces independent dense attention over 2048-position
dilated segments; outputs are normalized per (batch, head, channel) by the
sum over all segment positions, and divided by num_groups.

Sharding: 8 cores = 2 batches x 4 "head columns". Core c owns batch c//4 and
heads {j, 4+j, 8+j} where j = c%4 -- exactly 7 dense 2048x2048x64 attention
units per core (4 + 2 + 1 segments), perfectly balanced, with all segments of
any (batch, head) on one core so normalization needs no cross-core traffic.

Precision ("self-correcting f32r attention"): the reference's
x / x.sum(axis=(1,2)) normalization divides by a nearly-cancelling sum D,
which amplifies per-element noise ~300x -- but ONLY through D. Per-element
noise in x itself is unamplified, so the whole attention runs at reduced
precision and only D gets repaired:
  - P22 = f32r(64*exp(s)): ACT exp writes float32r (HW: round-to-nearest,
    11 mantissa bits). The PE consumes the same rounded values.
  - PV weights w22 = f32r(32*v), plus a denominator row of 32.0.
  - A matmul's cost is N cycles regardless of output partition count, so
    the 63 spare PSUM partitions of the PV matmul carry, FOR FREE, the
    w-rounding correction G[d] = sum_k (4096*(32v - w22))*P22 for channels
    0..62 (channel 63's correction is negligible in global L2 -- verified
    in simulation; 64+1+63 = 128 rows exactly fills the PSUM partition dim).
  - Host (f64): x = num/den;  D_d = sum_pos x_d + 2^-12 * sum_q r_q G[d,q];
    out = x / (3*D). The per-q r_q = 1/den_q makes G an essentially exact
    repair of the weight-rounding part of D's noise; the zero-mean P22
    rounding residual is left uncorrected (simulated total 2.9e-3 vs the
    2e-2 gate).
Scores feed the amplified path directly, so they keep k at ~fp32 via the
[kh;kl] K=128 stacking trick, with q at plain fp16 (the q-residual term was
simulated unnecessary): ONE fp16 matmul per unit. A ~160-col zero-weight
pad matmul keeps the PE stream denser than ACT's exp so the HAM clock-gate
stays at 8/8 (PE idling >~10% re-throttles the PE clock to 1.2GHz).

Device kernel per (chunk, k-block) unit (28 q-chunks of 512 x 16 k-blocks):
  S = khl_blk.T @ qhh (fp16, 512cyc) + 0-pad (160cyc) -> PSUM f32
  P22 = exp(S*ESC + ln64) -> SBUF f32r  (ACT, one batched pass per round)
  O[128, 512] += w22_blk.T @ P22 (f32r, 512cyc; rows: 64 num, 1 den, 63 G)
PE: ~2.3 streams/unit vs the 5-stream fp16-hi/lo baseline (510us). DVE only
copies chunk outputs. Engine budget per 3-unit round: PE ~1600ns (bottleneck
by design), ACT ~1490ns, DVE ~120ns.
"""

import os
import sys

if "/opt/trn_rl_repo" not in sys.path:
    sys.path.insert(0, "/opt/trn_rl_repo")
if "jax" not in sys.modules:
    os.environ.setdefault("JAX_PLATFORMS", "axon")

import math

import numpy as np

import concourse.bass as bass  # noqa: F401
import concourse.mybir as mybir
import concourse.tile as tile
from concourse import bacc
from concourse.bass_utils import run_bass_kernel_spmd

F32 = mybir.dt.float32
F32R = mybir.dt.float32r
F16 = mybir.dt.float16

B, N, H, D = 2, 8192, 12, 64
NSEG = 7           # segments per core
SEG = 2048         # dilated segment length
NCHUNK = NSEG * 4  # 512-wide q chunks per core
NKB = 16           # 128-row k blocks per segment
NUNIT = NCHUNK * NKB
RW = 3             # units per round (3 PSUM banks per score tile)
QSC = np.float32(256.0)               # fp16 pre-scale for Q/K
VSC = np.float32(32.0)                # pre-scale for V (and den row)
GSC = 4096.0                          # G-row scale: 2^12 * wres
ESC = float(0.125 / (256.0 * 256.0))  # exp scale: 1/sqrt(64) + descale
PBIAS = float(math.log(64.0))         # exp bias: P in [0.3, 16K]

_CACHE = {}
LAST_RESULT = {}


def _round_f32r(x):
    """Round f32 to the HW f32r grid (round-to-nearest, 11 mantissa bits)."""
    a = np.ascontiguousarray(x, np.float32)
    u = a.view(np.uint32).copy()
    u += np.uint32(1 << 11)
    u &= np.uint32(0xFFFFF000)
    return u.view(np.float32)


def _build_nc():
    nc = bacc.Bacc("TRN2", target_bir_lowering=False, debug=False,
                   enable_asserts=False, num_devices=8)
    qhh = nc.dram_tensor("qhh", [128, NSEG * SEG], F16, kind="ExternalInput")
    khl = nc.dram_tensor("khl", [128, NSEG * SEG], F16, kind="ExternalInput")
    w22 = nc.dram_tensor("w22", [128, NSEG * NKB * 128], F32R,
                         kind="ExternalInput")
    out = nc.dram_tensor("out", [128, NCHUNK * 512], F32,
                         kind="ExternalOutput")
    qhh_ap, khl_ap, w22_ap, out_ap = qhh.ap(), khl.ap(), w22.ap(), out.ap()

    with tile.TileContext(nc) as tc:
        with (
            tc.tile_pool(name="inp", bufs=1) as inp,
            tc.tile_pool(name="pt", bufs=5) as ptp,
            tc.tile_pool(name="osb", bufs=6) as osbp,
            tc.tile_pool(name="score", bufs=2, space="PSUM") as scp,
            tc.tile_pool(name="ot", bufs=2, space="PSUM") as otp,
        ):
            bias_t = inp.tile([128, 1], F32, tag="bias", name="bias_t")
            nc.vector.memset(bias_t[:, :], PBIAS)
            zpad = inp.tile([128, 128], F16, tag="zpad", name="zpad")
            nc.vector.memset(zpad[:, :], 0.0)

            # Warm-up prologue: runs while the input DMAs land. Dummy matmuls
            # keep the PE busy >3.4us so the HAM clock-gate opens before the
            # real rounds, and one dummy exp pulls in the ACT table load
            # (~2.7us) that would otherwise stall round 0.
            # fire the ACT exp-table load (~1.3us) immediately, before the
            # warm matmuls, so exp(round 0) runs without the table stall --
            # the smaller the initial ACT deficit, the smaller the one-time
            # PE catch-up gap at the PE->ACT pacing transition (a >2us lump
            # there trips the HAM MID window and costs a 3.4us half-clock dip)
            tjunk = inp.tile([128, 1], F32, tag="tjunk", name="tjunk")
            nc.scalar.activation(tjunk[:, :], bias_t[:, :],
                                 mybir.ActivationFunctionType.Exp, scale=1.0,
                                 bias=bias_t[:, :])
            wsrc = inp.tile([128, 128], F16, tag="wsrc", name="wsrc")
            wjunk = inp.tile([128, 512], F16, tag="wjunk", name="wjunk")
            nc.vector.memset(wsrc[:, :], 0.01)
            nc.vector.memset(wjunk[:, :], 0.01)
            warm = scp.tile([128, 512 * RW], F32, tag="score", name="warm")
            for i in range(14):
                nc.tensor.matmul(warm[:, (i % 3) * 512:(i % 3 + 1) * 512],
                                 wsrc[:, :], wjunk[:, :],
                                 start=(i < 3), stop=(i >= 11))
            wp = ptp.tile([128, 512 * RW], F32R, tag="p22", name="warmp")
            nc.scalar.activation(
                wp[:, :512], warm[:, :512],
                mybir.ActivationFunctionType.Exp, scale=ESC, bias=bias_t[:, :])

            qh_sb, k_sb, w_sb = [], [], []
            for s in range(NSEG):
                qh = inp.tile([128, SEG], F16, tag=f"qh{s}", name=f"qh{s}")
                kk = inp.tile([128, SEG], F16, tag=f"k{s}", name=f"k{s}")
                wv = inp.tile([128, NKB * 128], F32R, tag=f"wv{s}",
                              name=f"wv{s}")
                # split the first segment's Q/K transfers across DMA queues so
                # round 0 isn't gated on a single ~512KB queue transfer
                nsl_dma = 4 if s == 0 else 1
                for t, ap_ in ((qh, qhh_ap), (kk, khl_ap)):
                    step = SEG // nsl_dma
                    for z in range(nsl_dma):
                        lo = z * step
                        nc.sync.dma_start(
                            t[:, lo:lo + step],
                            ap_[:, s * SEG + lo:s * SEG + lo + step])
                nc.sync.dma_start(
                    wv[:, :], w22_ap[:, s * NKB * 128:(s + 1) * NKB * 128])
                qh_sb.append(qh)
                k_sb.append(kk)
                w_sb.append(wv)

            ot_tiles = {}
            pend1, pend2 = [], []  # PV work lagged by 1 and 2 rounds

            def flush(items):
                for p22ref, i, u in items:
                    cid, kb = divmod(u, NKB)
                    s = cid // 4
                    if kb == 0:
                        ot_tiles[cid] = otp.tile([128, 512], F32, tag="ot",
                                                 name=f"ot{cid}")
                    vsl = slice(kb * 128, (kb + 1) * 128)
                    psl = slice(i * 512, (i + 1) * 512)
                    nc.tensor.matmul(ot_tiles[cid][:, :], w_sb[s][:, vsl],
                                     p22ref[:, psl],
                                     start=(kb == 0), stop=(kb == NKB - 1))
                    if kb == NKB - 1:
                        o_sb = osbp.tile([128, 512], F32, tag="osb",
                                         name=f"osb{cid}")
                        nc.vector.tensor_copy(o_sb[:, :], ot_tiles[cid][:, :])
                        nc.sync.dma_start(
                            out_ap[:, cid * 512:(cid + 1) * 512], o_sb[:, :])

            for r in range((NUNIT + RW - 1) // RW):
                units = range(r * RW, min((r + 1) * RW, NUNIT))
                nu = len(units)
                score = scp.tile([128, 512 * RW], F32, tag="score",
                                 name=f"score{r}")
                for i, u in enumerate(units):
                    cid, kb = divmod(u, NKB)
                    s, c = divmod(cid, 4)
                    osl = slice(i * 512, (i + 1) * 512)
                    csl = slice(c * 512, (c + 1) * 512)
                    ksl = slice(kb * 128, (kb + 1) * 128)
                    nc.tensor.matmul(score[:, osl], k_sb[s][:, ksl],
                                     qh_sb[s][:, csl], start=True, stop=True)
                p22 = ptp.tile([128, 512 * RW], F32R, tag="p22",
                               name=f"p22_{r}")
                nsl = slice(0, 512 * nu)
                nc.scalar.activation(
                    p22[:, nsl], score[:, nsl],
                    mybir.ActivationFunctionType.Exp, scale=ESC,
                    bias=bias_t[:, :])
                if r < 2:
                    # startup filler: keep the PE streaming through the
                    # pipe-fill with dummies aimed at an OT-pool slot.
                    fill = otp.tile([128, 512], F32, tag="ot", name=f"fill{r}")
                    for z in range(5):
                        nc.tensor.matmul(fill[:, :], wsrc[:, :], wjunk[:, :],
                                         start=(z == 0), stop=(z == 4))
                flush(pend2)
                pend2 = pend1
                pend1 = [(p22, i, u) for i, u in enumerate(units)]
            flush(pend2)
            flush(pend1)

    nc.compile()
    return nc


def _gather_segs(query, key, value, core):
    b, j = divmod(core, 4)
    segs = []
    for arr in (query, key, value):
        h0 = arr[b, :, j, :].reshape(4, SEG, D)
        h1 = arr[b, :, 4 + j, :].reshape(2, 4096, D)[:, 1::2, :]
        h2 = arr[b, 2::4, 8 + j, :][None]
        segs.append(np.concatenate([h0, h1, h2], axis=0))  # [7, 2048, 64]
    return segs


def _prep_core(query, key, value, core):
    qs, ks, vs = _gather_segs(query, key, value, core)
    # [64, NSEG*SEG] with col = s*SEG + p
    qt = (qs * QSC).transpose(2, 0, 1).reshape(D, NSEG * SEG)
    kt = (ks * QSC).transpose(2, 0, 1).reshape(D, NSEG * SEG)
    qh = qt.astype(np.float16)
    kh = kt.astype(np.float16)
    kl = (kt - kh).astype(np.float16)
    v32 = (vs * VSC).astype(np.float32)            # [7, 2048, 64]
    w22v = _round_f32r(v32)
    wres = _round_f32r(GSC * (v32.astype(np.float64)
                              - w22v.astype(np.float64)).astype(np.float32))
    # per (seg, kb) block [128 kpos, 128 outrows]:
    #   cols 0:64 = w22v, col 64 = 32.0, cols 65:128 = G weights (ch 0..62)
    wblk = np.empty((NSEG, SEG, 128), np.float32)
    wblk[:, :, :64] = w22v
    wblk[:, :, 64] = float(VSC)
    wblk[:, :, 65:] = wres[:, :, :63]
    w22 = (wblk.reshape(NSEG, NKB, 128, 128).transpose(2, 0, 1, 3)
           .reshape(128, -1))
    return {
        "qhh": np.ascontiguousarray(np.concatenate([qh, qh], axis=0)),
        "khl": np.ascontiguousarray(np.concatenate([kh, kl], axis=0)),
        "w22": np.ascontiguousarray(w22),
    }


def _unshard(results, dtype):
    full = np.zeros((B, N, H, D), dtype)
    groups = [(0, 4), (4, 6), (6, 7)]
    for core in range(8):
        b, j = divmod(core, 4)
        o = results[core]["out"].astype(np.float64)
        num, den, G = o[:64], o[64], o[65:]          # [64|63, 14336], [14336]
        r = 1.0 / den
        x = num * r[None, :]
        for g0, g1 in groups:
            gcols = slice(g0 * SEG, g1 * SEG)
            Dv = x[:, gcols].sum(axis=1)             # [64]
            C = np.zeros(64)
            C[:63] = (G[:, gcols] * r[None, gcols]).sum(axis=1) / GSC
            x[:, gcols] = x[:, gcols] / (3.0 * (Dv + C))[:, None]
        h0 = x[:, :4 * SEG]
        full[b, :, j, :] = h0.T
        h1 = x[:, 4 * SEG:6 * SEG]
        for g in range(2):
            full[b, g * 4096 + 1:(g + 1) * 4096:2, 4 + j, :] = \
                h1[:, g * SEG:(g + 1) * SEG].T
        full[b, 2::4, 8 + j, :] = x[:, 6 * SEG:].T
    return full


def _consistent(results):
    for core in range(8):
        den = results[core]["out"][64].astype(np.float64)
        if not np.isfinite(den).all() or (den <= 0).any():
            return False
        # den = 32 * sum_k P22 over 2048 rows, P22 in [0.3, 16K]:
        # sane bounds catch corrupted/partial runs
        if den.min() < 32 * 2048 * 0.01 or den.max() > 32 * 2048 * 2e4:
            return False
    return True


def _ensure_axon_backend():
    """The bass PJRT path needs the axon/neuron jax backend. A harness may
    pin JAX_PLATFORMS=cpu for its reference; re-select axon if so."""
    import jax
    try:
        plat = jax.devices()[0].platform
    except Exception:
        plat = ""
    if plat not in ("axon", "neuron"):
        try:
            jax.config.update("jax_platforms", "axon,cpu")
            jax.devices()
        except Exception:
            pass


def kernel(query, key, value):
    _ensure_axon_backend()
    query = np.asarray(query, np.float32)
    key = np.asarray(key, np.float32)
    value = np.asarray(value, np.float32)
    assert query.shape == (B, N, H, D)

    if "nc" not in _CACHE:
        _CACHE["nc"] = _build_nc()
    nc = _CACHE["nc"]

    in_maps = [_prep_core(query, key, value, c) for c in range(8)]
    res = run_bass_kernel_spmd(nc, in_maps, core_ids=list(range(8)))
    if not _consistent(res.results):
        # transient first-execution flakes have been observed once; both
        # checks can only fail on a corrupted run, so retry once.
        res = run_bass_kernel_spmd(nc, in_maps, core_ids=list(range(8)))
    LAST_RESULT["exec_time_ns"] = res.exec_time_ns
    LAST_RESULT["results"] = res.results
    return _unshard(res.results, query.dtype)


# revision 15
# speedup vs baseline: 1.1774x; 1.0009x over previous
"""Dilated attention (LongNet-style) Trainium2 kernel, v4.

Problem: query/key/value (2, 8192, 12, 64) f32. Three dilation groups
(segment lengths 2048/4096/8192, dilation 1/2/4, head slices 0:4/4:8/8:12).
Each group's gather produces independent dense attention over 2048-position
dilated segments; outputs are normalized per (batch, head, channel) by the
sum over all segment positions, and divided by num_groups.

Sharding: 8 cores = 2 batches x 4 "head columns". Core c owns batch c//4 and
heads {j, 4+j, 8+j} where j = c%4 -- exactly 7 dense 2048x2048x64 attention
units per core (4 + 2 + 1 segments), perfectly balanced, with all segments of
any (batch, head) on one core so normalization needs no cross-core traffic.

Precision ("self-correcting f32r attention"): the reference's
x / x.sum(axis=(1,2)) normalization divides by a nearly-cancelling sum D,
which amplifies per-element noise ~300x -- but ONLY through D. Per-element
noise in x itself is unamplified, so the whole attention runs at reduced
precision and only D gets repaired:
  - P22 = f32r(64*exp(s)): ACT exp writes float32r (HW: round-to-nearest,
    11 mantissa bits). The PE consumes the same rounded values.
  - PV weights w22 = f32r(32*v), plus a denominator row of 32.0.
  - A matmul's cost is N cycles regardless of output partition count, so
    the 63 spare PSUM partitions of the PV matmul carry, FOR FREE, the
    w-rounding correction G[d] = sum_k (4096*(32v - w22))*P22 for channels
    0..62 (channel 63's correction is negligible in global L2 -- verified
    in simulation; 64+1+63 = 128 rows exactly fills the PSUM partition dim).
  - Host (f64): x = num/den;  D_d = sum_pos x_d + 2^-12 * sum_q r_q G[d,q];
    out = x / (3*D). The per-q r_q = 1/den_q makes G an essentially exact
    repair of the weight-rounding part of D's noise; the zero-mean P22
    rounding residual is left uncorrected (simulated total 2.9e-3 vs the
    2e-2 gate).
Scores feed the amplified path directly, so they keep k at ~fp32 via the
[kh;kl] K=128 stacking trick, with q at plain fp16 (the q-residual term was
simulated unnecessary): ONE fp16 matmul per unit.

Device kernel per (chunk, k-block) unit (28 q-chunks of 512 x 16 k-blocks):
  S = khl_blk.T @ qhh (fp16, 512cyc) -> PSUM f32
  P22 = exp(S*ESC + ln64) -> SBUF f32r  (ACT, one batched pass per round)
  O[128, 512] += w22_blk.T @ P22 (f32r, 512cyc; rows: 64 num, 1 den, 63 G)
PE: 2 streams/unit vs the 5-stream fp16-hi/lo baseline (510us); the ACT exp
(~1540ns per 3-unit round) paces the steady state with the PE ~7% idle --
tolerated by the HAM clock-gate, which holds 8/8 for the whole run given a
primed ACT table (early dummy exp) and startup fills (measured: zero
re-throttles). DVE only copies chunk outputs. 236us HW exec on 8 cores,
rel err 2.0e-3 (gate 2e-2).
"""

import os
import sys

if "/opt/trn_rl_repo" not in sys.path:
    sys.path.insert(0, "/opt/trn_rl_repo")
if "jax" not in sys.modules:
    os.environ.setdefault("JAX_PLATFORMS", "axon")

import math

import numpy as np

import concourse.bass as bass  # noqa: F401
import concourse.mybir as mybir
import concourse.tile as tile
from concourse import bacc
from concourse.bass_utils import run_bass_kernel_spmd

F32 = mybir.dt.float32
F32R = mybir.dt.float32r
F16 = mybir.dt.float16

B, N, H, D = 2, 8192, 12, 64
NSEG = 7           # segments per core
SEG = 2048         # dilated segment length
NCHUNK = NSEG * 4  # 512-wide q chunks per core
NKB = 16           # 128-row k blocks per segment
NUNIT = NCHUNK * NKB
RW = 3             # units per round (3 PSUM banks per score tile)
QSC = np.float32(256.0)               # fp16 pre-scale for Q/K
VSC = np.float32(32.0)                # pre-scale for V (and den row)
GSC = 4096.0                          # G-row scale: 2^12 * wres
ESC = float(0.125 / (256.0 * 256.0))  # exp scale: 1/sqrt(64) + descale
PBIAS = float(math.log(64.0))         # exp bias: P in [0.3, 16K]

_CACHE = {}
LAST_RESULT = {}


def _round_f32r(x):
    """Round f32 to the HW f32r grid (round-to-nearest, 11 mantissa bits)."""
    a = np.ascontiguousarray(x, np.float32)
    u = a.view(np.uint32).copy()
    u += np.uint32(1 << 11)
    u &= np.uint32(0xFFFFF000)
    return u.view(np.float32)


def _build_nc():
    nc = bacc.Bacc("TRN2", target_bir_lowering=False, debug=False,
                   enable_asserts=False, num_devices=8)
    qhh = nc.dram_tensor("qhh", [128, NSEG * SEG], F16, kind="ExternalInput")
    khl = nc.dram_tensor("khl", [128, NSEG * SEG], F16, kind="ExternalInput")
    w22 = nc.dram_tensor("w22", [128, NSEG * NKB * 128], F32R,
                         kind="ExternalInput")
    out = nc.dram_tensor("out", [128, NCHUNK * 512], F32,
                         kind="ExternalOutput")
    qhh_ap, khl_ap, w22_ap, out_ap = qhh.ap(), khl.ap(), w22.ap(), out.ap()

    with tile.TileContext(nc) as tc:
        with (
            tc.tile_pool(name="inp", bufs=1) as inp,
            tc.tile_pool(name="pt", bufs=5) as ptp,
            tc.tile_pool(name="osb", bufs=6) as osbp,
            tc.tile_pool(name="score", bufs=2, space="PSUM") as scp,
            tc.tile_pool(name="ot", bufs=2, space="PSUM") as otp,
        ):
            bias_t = inp.tile([128, 1], F32, tag="bias", name="bias_t")
            nc.vector.memset(bias_t[:, :], PBIAS)

            # Warm-up prologue: runs while the input DMAs land. Dummy matmuls
            # keep the PE busy >3.4us so the HAM clock-gate opens before the
            # real rounds, and one dummy exp pulls in the ACT table load
            # (~2.7us) that would otherwise stall round 0.
            # fire the ACT exp-table load (~1.3us) immediately, before the
            # warm matmuls, so exp(round 0) runs without the table stall --
            # the smaller the initial ACT deficit, the smaller the one-time
            # PE catch-up gap at the PE->ACT pacing transition (a >2us lump
            # there trips the HAM MID window and costs a 3.4us half-clock dip)
            tjunk = inp.tile([128, 1], F32, tag="tjunk", name="tjunk")
            nc.scalar.activation(tjunk[:, :], bias_t[:, :],
                                 mybir.ActivationFunctionType.Exp, scale=1.0,
                                 bias=bias_t[:, :])
            wsrc = inp.tile([128, 128], F16, tag="wsrc", name="wsrc")
            wjunk = inp.tile([128, 512], F16, tag="wjunk", name="wjunk")
            nc.vector.memset(wsrc[:, :], 0.01)
            nc.vector.memset(wjunk[:, :], 0.01)
            warm = scp.tile([128, 512 * RW], F32, tag="score", name="warm")
            for i in range(14):
                nc.tensor.matmul(warm[:, (i % 3) * 512:(i % 3 + 1) * 512],
                                 wsrc[:, :], wjunk[:, :],
                                 start=(i < 3), stop=(i >= 11))
            wp = ptp.tile([128, 512 * RW], F32R, tag="p22", name="warmp")
            nc.scalar.activation(
                wp[:, :512], warm[:, :512],
                mybir.ActivationFunctionType.Exp, scale=ESC, bias=bias_t[:, :])

            qh_sb, k_sb, w_sb = [], [], []
            for s in range(NSEG):
                qh = inp.tile([128, SEG], F16, tag=f"qh{s}", name=f"qh{s}")
                kk = inp.tile([128, SEG], F16, tag=f"k{s}", name=f"k{s}")
                wv = inp.tile([128, NKB * 128], F32R, tag=f"wv{s}",
                              name=f"wv{s}")
                # split the first segment's Q/K transfers across DMA queues so
                # round 0 isn't gated on a single ~512KB queue transfer
                nsl_dma = 4 if s == 0 else 1
                for t, ap_ in ((qh, qhh_ap), (kk, khl_ap)):
                    step = SEG // nsl_dma
                    for z in range(nsl_dma):
                        lo = z * step
                        nc.sync.dma_start(
                            t[:, lo:lo + step],
                            ap_[:, s * SEG + lo:s * SEG + lo + step])
                nc.sync.dma_start(
                    wv[:, :], w22_ap[:, s * NKB * 128:(s + 1) * NKB * 128])
                qh_sb.append(qh)
                k_sb.append(kk)
                w_sb.append(wv)

            ot_tiles = {}
            pend1, pend2 = [], []  # PV work lagged by 1 and 2 rounds

            def flush(items):
                for p22ref, i, u in items:
                    cid, kb = divmod(u, NKB)
                    s = cid // 4
                    if kb == 0:
                        ot_tiles[cid] = otp.tile([128, 512], F32, tag="ot",
                                                 name=f"ot{cid}")
                    vsl = slice(kb * 128, (kb + 1) * 128)
                    psl = slice(i * 512, (i + 1) * 512)
                    nc.tensor.matmul(ot_tiles[cid][:, :], w_sb[s][:, vsl],
                                     p22ref[:, psl],
                                     start=(kb == 0), stop=(kb == NKB - 1))
                    if kb == NKB - 1:
                        o_sb = osbp.tile([128, 512], F32, tag="osb",
                                         name=f"osb{cid}")
                        nc.vector.tensor_copy(o_sb[:, :], ot_tiles[cid][:, :])
                        nc.sync.dma_start(
                            out_ap[:, cid * 512:(cid + 1) * 512], o_sb[:, :])

            for r in range((NUNIT + RW - 1) // RW):
                units = range(r * RW, min((r + 1) * RW, NUNIT))
                nu = len(units)
                score = scp.tile([128, 512 * RW], F32, tag="score",
                                 name=f"score{r}")
                for i, u in enumerate(units):
                    cid, kb = divmod(u, NKB)
                    s, c = divmod(cid, 4)
                    osl = slice(i * 512, (i + 1) * 512)
                    csl = slice(c * 512, (c + 1) * 512)
                    ksl = slice(kb * 128, (kb + 1) * 128)
                    nc.tensor.matmul(score[:, osl], k_sb[s][:, ksl],
                                     qh_sb[s][:, csl], start=True, stop=True)
                p22 = ptp.tile([128, 512 * RW], F32R, tag="p22",
                               name=f"p22_{r}")
                nsl = slice(0, 512 * nu)
                nc.scalar.activation(
                    p22[:, nsl], score[:, nsl],
                    mybir.ActivationFunctionType.Exp, scale=ESC,
                    bias=bias_t[:, :])
                if r < 2:
                    # startup filler: keep the PE streaming through the
                    # pipe-fill with dummies aimed at an OT-pool slot.
                    fill = otp.tile([128, 512], F32, tag="ot", name=f"fill{r}")
                    for z in range(5):
                        nc.tensor.matmul(fill[:, :], wsrc[:, :], wjunk[:, :],
                                         start=(z == 0), stop=(z == 4))
                flush(pend2)
                pend2 = pend1
                pend1 = [(p22, i, u) for i, u in enumerate(units)]
            flush(pend2)
            flush(pend1)

    nc.compile()
    return nc


def _gather_segs(query, key, value, core):
    b, j = divmod(core, 4)
    segs = []
    for arr in (query, key, value):
        h0 = arr[b, :, j, :].reshape(4, SEG, D)
        h1 = arr[b, :, 4 + j, :].reshape(2, 4096, D)[:, 1::2, :]
        h2 = arr[b, 2::4, 8 + j, :][None]
        segs.append(np.concatenate([h0, h1, h2], axis=0))  # [7, 2048, 64]
    return segs


def _prep_core(query, key, value, core):
    qs, ks, vs = _gather_segs(query, key, value, core)
    # [64, NSEG*SEG] with col = s*SEG + p
    qt = (qs * QSC).transpose(2, 0, 1).reshape(D, NSEG * SEG)
    kt = (ks * QSC).transpose(2, 0, 1).reshape(D, NSEG * SEG)
    qh = qt.astype(np.float16)
    kh = kt.astype(np.float16)
    kl = (kt - kh).astype(np.float16)
    v32 = (vs * VSC).astype(np.float32)            # [7, 2048, 64]
    w22v = _round_f32r(v32)
    wres = _round_f32r(GSC * (v32.astype(np.float64)
                              - w22v.astype(np.float64)).astype(np.float32))
    # per (seg, kb) block [128 kpos, 128 outrows]:
    #   cols 0:64 = w22v, col 64 = 32.0, cols 65:128 = G weights (ch 0..62)
    wblk = np.empty((NSEG, SEG, 128), np.float32)
    wblk[:, :, :64] = w22v
    wblk[:, :, 64] = float(VSC)
    wblk[:, :, 65:] = wres[:, :, :63]
    w22 = (wblk.reshape(NSEG, NKB, 128, 128).transpose(2, 0, 1, 3)
           .reshape(128, -1))
    return {
        "qhh": np.ascontiguousarray(np.concatenate([qh, qh], axis=0)),
        "khl": np.ascontiguousarray(np.concatenate([kh, kl], axis=0)),
        "w22": np.ascontiguousarray(w22),
    }


def _unshard(results, dtype):
    full = np.zeros((B, N, H, D), dtype)
    groups = [(0, 4), (4, 6), (6, 7)]
    for core in range(8):
        b, j = divmod(core, 4)
        o = results[core]["out"].astype(np.float64)
        num, den, G = o[:64], o[64], o[65:]          # [64|63, 14336], [14336]
        r = 1.0 / den
        x = num * r[None, :]
        for g0, g1 in groups:
            gcols = slice(g0 * SEG, g1 * SEG)
            Dv = x[:, gcols].sum(axis=1)             # [64]
            C = np.zeros(64)
            C[:63] = (G[:, gcols] * r[None, gcols]).sum(axis=1) / GSC
            x[:, gcols] = x[:, gcols] / (3.0 * (Dv + C))[:, None]
        h0 = x[:, :4 * SEG]
        full[b, :, j, :] = h0.T
        h1 = x[:, 4 * SEG:6 * SEG]
        for g in range(2):
            full[b, g * 4096 + 1:(g + 1) * 4096:2, 4 + j, :] = \
                h1[:, g * SEG:(g + 1) * SEG].T
        full[b, 2::4, 8 + j, :] = x[:, 6 * SEG:].T
    return full


def _consistent(results):
    for core in range(8):
        den = results[core]["out"][64].astype(np.float64)
        if not np.isfinite(den).all() or (den <= 0).any():
            return False
        # den = 32 * sum_k P22 over 2048 rows, P22 in [0.3, 16K]:
        # sane bounds catch corrupted/partial runs
        if den.min() < 32 * 2048 * 0.01 or den.max() > 32 * 2048 * 2e4:
            return False
    return True


def _ensure_axon_backend():
    """The bass PJRT path needs the axon/neuron jax backend. A harness may
    pin JAX_PLATFORMS=cpu for its reference; re-select axon if so."""
    import jax
    try:
        plat = jax.devices()[0].platform
    except Exception:
        plat = ""
    if plat not in ("axon", "neuron"):
        try:
            jax.config.update("jax_platforms", "axon,cpu")
            jax.devices()
        except Exception:
            pass


def kernel(query, key, value):
    _ensure_axon_backend()
    query = np.asarray(query, np.float32)
    key = np.asarray(key, np.float32)
    value = np.asarray(value, np.float32)
    assert query.shape == (B, N, H, D)

    if "nc" not in _CACHE:
        _CACHE["nc"] = _build_nc()
    nc = _CACHE["nc"]

    in_maps = [_prep_core(query, key, value, c) for c in range(8)]
    res = run_bass_kernel_spmd(nc, in_maps, core_ids=list(range(8)))
    if not _consistent(res.results):
        # transient first-execution flakes have been observed once; both
        # checks can only fail on a corrupted run, so retry once.
        res = run_bass_kernel_spmd(nc, in_maps, core_ids=list(range(8)))
    LAST_RESULT["exec_time_ns"] = res.exec_time_ns
    LAST_RESULT["results"] = res.results
    return _unshard(res.results, query.dtype)
